# revision 1
# baseline (speedup 1.0000x reference)
"""Causal multi-head self-attention on 8 Trainium2 NeuronCores.

Problem: x[2,2048,1024], 16 heads x 64 dims, causal softmax attention,
four 1024x1024 projections (q,k,v,o), fp32.

Sharding (hardcoded): core c in 0..7 handles batch b=c//4 and the 4-head
group g=c%4 (heads 4g..4g+3).  Data-parallel over B, tensor-parallel over
heads.  Each core computes its heads' attention contribution projected
through its slice of wo; the host sums the 4 partial outputs per batch.

Device dataflow is fully "transposed" so no on-chip transposes are needed:
  qT = (wq_rows/8) @ x_b.T        [256,2048]   (scale 1/sqrt(64) folded in)
  kT =  wk_rows    @ x_b.T        [256,2048]
  V  =  x_b @ wv_rows.T           [2048,256]   (+ ones column per head)
  sT = k_chunk @ qT_h             [tk,tq] tiles; causal tiles only, and
                                  diagonal tiles only over their live columns
  pT = exp(sT); causally-invalid triangle zeroed in place on GPSIMD
                (affine_select), so softmax needs no additive mask and no
                row-max pass (scores are bounded ~|10| for this problem)
  [oT;den] = [V_h|1].T @ pT       (ones column gives softmax denominator)
  aT = oT * (1/den)               (1/den broadcast across partitions on GPSIMD)
  y_partial = aT.T @ woT_cols     [2048,1024]
All matmuls run as float32r (full-rate fp32 on the PE at N>=256, ~12-bit
mantissa, end-to-end rel err ~2e-4).  Emission order pipelines DMA-in,
projections, attention rounds, and per-round output projection + DMA-out
so PE/ACT/DVE/GPSIMD/DMA overlap; modeled single-core time ~156us.
"""

import sys

sys.path.insert(0, "/opt/trn_rl_repo")

import numpy as np

import concourse.mybir as mybir
import concourse.tile as tile
from concourse import bacc, bass_utils

B, T, C = 2, 2048, 1024
H, D = 16, 64
NCORES = 8
HG = 4            # heads per core
DH = HG * D       # 256 projected dims per core
NK = C // 128     # 8 contraction chunks over C
NTQ = T // 512    # 4 query-column chunks
NM = T // 128     # 16 row chunks of T
F32 = mybir.dt.float32
F32R = mybir.dt.float32r
EXP = mybir.ActivationFunctionType.Exp


def build_program(nc):
    xt_d = nc.dram_tensor("xt", [C, T], F32R, kind="ExternalInput")
    wqt_d = nc.dram_tensor("wqt", [C, DH], F32R, kind="ExternalInput")
    wkt_d = nc.dram_tensor("wkt", [C, DH], F32R, kind="ExternalInput")
    wvt_d = nc.dram_tensor("wvt", [C, DH], F32R, kind="ExternalInput")
    wot_d = nc.dram_tensor("wot", [DH, C], F32R, kind="ExternalInput")
    y_d = nc.dram_tensor("y", [T, C], F32, kind="ExternalOutput")
    xt, wqt, wkt, wvt, wot, y = (
        xt_d.ap(), wqt_d.ap(), wkt_d.ap(), wvt_d.ap(), wot_d.ap(), y_d.ap())

    with nc.allow_low_precision(reason="fp32r matmul dataflow"), \
            tile.TileContext(nc) as tc:
        with (
            tc.tile_pool(name="big", bufs=1) as big,
            tc.tile_pool(name="work", bufs=6) as work,
            tc.tile_pool(name="ps", bufs=2, space="PSUM") as ps,
            tc.tile_pool(name="ps2", bufs=2, space="PSUM") as ps2,
            tc.tile_pool(name="psav", bufs=2, space="PSUM") as psav,
        ):
            # ---- persistent SBUF tensors ----
            xt_s = big.tile([128, NK, T], F32R, tag="xt")
            wq_s = big.tile([128, NK, DH], F32R, tag="wq")
            wk_s = big.tile([128, NK, DH], F32R, tag="wk")
            wv_s = big.tile([128, NK, DH], F32R, tag="wv")
            wo_s = big.tile([128, 2, C], F32R, tag="wo")
            qt_s = big.tile([128, 2, T], F32R, tag="qt")
            kt_s = big.tile([128, 2, T], F32R, tag="kt")
            va_s = big.tile([128, NM, HG, D + 1], F32R, tag="va")
            at_s = big.tile([128, 2, T], F32R, tag="at")
            onesc = big.tile([128, 64], F32, tag="onesc")

            # ---- constants: ones columns for V_aug (softmax denominator) ----
            nc.gpsimd.memset(onesc[:], 1.0)
            nc.vector.tensor_copy(
                va_s[:, :, :, D], onesc.rearrange("p (a b) -> p a b", a=NM))
            # touch Exp during the DMA-bound startup so the ACT function
            # table is resident before the first real softmax tile
            warm = work.tile([1, 32], F32, tag="warm", bufs=1)
            nc.scalar.activation(warm[:], onesc[0:1, 0:32], EXP)

            def xt_dma(n):
                cs = slice(512 * n, 512 * (n + 1))
                for k in range(NK):
                    nc.sync.dma_start(xt_s[:, k, cs],
                                      xt[128 * k:128 * (k + 1), cs])

            # ---- q (or k) projection for one x.T column block ----
            def proj_half(n, w_s, out_s, lbl):
                cs = slice(512 * n, 512 * (n + 1))
                for m in range(2):
                    msl = slice(128 * m, 128 * (m + 1))
                    pq = ps.tile([128, 512], F32, tag="mm",
                                 name=f"p{lbl}_{n}_{m}")
                    for k in range(NK):
                        nc.tensor.matmul(pq[:], (w_s[:, k, msl]),
                                         (xt_s[:, k, cs]),
                                         start=(k == 0), stop=(k == NK - 1))
                    nc.scalar.copy(out_s[:, m, cs], pq[:])

            def proj_n(n):
                proj_half(n, wq_s, qt_s, "q")
                proj_half(n, wk_s, kt_s, "k")

            # q weights + x block 0 + k weights first; block-0 projections
            # start while x blocks 1..3 stream in.
            for k in range(NK):
                nc.sync.dma_start(wq_s[:, k], wqt[128 * k:128 * (k + 1)])
            xt_dma(0)
            for k in range(NK):
                nc.sync.dma_start(wk_s[:, k], wkt[128 * k:128 * (k + 1)])
            proj_n(0)
            xt_dma(1)

            # ---- V projection chunk (natural layout, writes V_aug) ----
            def v_chunk(m):
                msl = slice(128 * m, 128 * (m + 1))
                pv = ps.tile([128, DH], F32, tag="mm", name=f"pv{m}")
                for k in range(NK):
                    nc.tensor.matmul(pv[:], (xt_s[:, k, msl]), (wv_s[:, k]),
                                     start=(k == 0), stop=(k == NK - 1))
                nc.vector.tensor_copy(
                    va_s[:, m, :, 0:D], pv.rearrange("p (g d) -> p g d", g=HG))

            # ---- attention group (head h, query block j); causal tiles ----
            def attn(h, j):
                ht = h // 2
                ho = (h % 2) * 64
                ni = 4 * j + 4  # tk chunks 0..4j+3 are causal-relevant
                kq = lambda i, lo, w: (
                    kt_s[ho:ho + 64, ht, 128 * i:128 * (i + 1)],
                    qt_s[ho:ho + 64, ht, 512 * j + lo:512 * j + lo + w])
                pts = []  # (rhs_ap, lo) per chunk i, for the AV accumulation
                # full tiles pairwise: one 2-bank PSUM + one wide exp
                for a in range(0, 4 * j, 2):
                    pst2 = ps2.tile([128, 1024], F32, tag="mm2",
                                    name=f"pst2_{h}_{j}_{a}")
                    for half in range(2):
                        kk_, qq = kq(a + half, 0, 512)
                        nc.tensor.matmul(pst2[:, 512 * half:512 * (half + 1)],
                                         kk_, qq, start=True, stop=True)
                    pt2 = work.tile([128, 1024], F32R, tag="pt2", bufs=4,
                                    name=f"pt2_{h}_{j}_{a}")
                    nc.scalar.activation(pt2[:], pst2[:], EXP)
                    pts.append((pt2[:, 0:512], 0))
                    pts.append((pt2[:, 512:1024], 0))
                # diagonal tiles r=0..3: columns >= 128r+p are valid; compute
                # only [lo, 512) with lo = min(128r, 256) (fp32r wants N>=256).
                # r=0,1 each get their own tile; r=2,3 (both 256 wide) share
                # one PSUM tile and one exp.
                # r=0 ([0:512)) and r=1 (live cols [128:512), packed at
                # [512:896)) share one 2-bank PSUM and one 896-wide exp
                pst01 = ps2.tile([128, 1024], F32, tag="mm2",
                                 name=f"pst01_{h}_{j}")
                kk_, qq = kq(4 * j, 0, 512)
                nc.tensor.matmul(pst01[:, 0:512], kk_, qq, start=True, stop=True)
                kk_, qq = kq(4 * j + 1, 128, 384)
                nc.tensor.matmul(pst01[:, 512:896], kk_, qq, start=True, stop=True)
                pt01 = work.tile([128, 1024], F32R, tag="pt2", bufs=4,
                                 name=f"pt01_{h}_{j}")
                nc.scalar.activation(pt01[:, 0:896], pst01[:, 0:896], EXP)
                # invalid entries only occur in the first 128 columns of each
                # region — zero just those bands
                nc.gpsimd.affine_select(
                    out=pt01[:, 0:128], in_=pt01[:, 0:128],
                    compare_op=mybir.AluOpType.is_ge,
                    fill=0.0, base=0,
                    pattern=[[1, 128]], channel_multiplier=-1)
                nc.gpsimd.affine_select(
                    out=pt01[:, 512:640], in_=pt01[:, 512:640],
                    compare_op=mybir.AluOpType.is_ge,
                    fill=0.0, base=0,
                    pattern=[[1, 128]], channel_multiplier=-1)
                pts.append((pt01[:, 0:512], 0))
                pts.append((pt01[:, 512:896], 128))
                pstd = ps.tile([128, 512], F32, tag="mm",
                               name=f"pstd_{h}_{j}")
                for r in (2, 3):
                    kk_, qq = kq(4 * j + r, 256, 256)
                    nc.tensor.matmul(pstd[:, 256 * (r - 2):256 * (r - 1)],
                                     kk_, qq, start=True, stop=True)
                ptd = work.tile([128, 512], F32R, tag="pt", bufs=6,
                                name=f"ptd_{h}_{j}")
                nc.scalar.activation(ptd[:], pstd[:], EXP)
                # r=2 half holds tq=256+f: invalid only for f < p (first 128
                # cols); r=3 half holds tq=256+u: invalid for u < 128+p (can
                # span the whole half)
                nc.gpsimd.affine_select(
                    out=ptd[:, 0:128], in_=ptd[:, 0:128],
                    compare_op=mybir.AluOpType.is_ge,
                    fill=0.0, base=0,
                    pattern=[[1, 128]], channel_multiplier=-1)
                pts.append((ptd[:, 0:256], 256))
                nc.gpsimd.affine_select(
                    out=ptd[:, 256:512], in_=ptd[:, 256:512],
                    compare_op=mybir.AluOpType.is_ge,
                    fill=0.0, base=-128,
                    pattern=[[1, 256]], channel_multiplier=-1)
                pts.append((ptd[:, 256:512], 256))
                pav = psav.tile([D + 1, 512], F32, tag="av",
                                name=f"pav_{h}_{j}")
                for i in range(ni):
                    rhs, lo = pts[i]
                    nc.tensor.matmul(pav[:, lo:], (va_s[:, i, h]), rhs,
                                     start=(i == 0), stop=(i == ni - 1))
                # normalize: oT[d,tq] / den[tq] (partition-broadcast on gpsimd
                # keeps the PE stream free of tiny recip-gated matmuls)
                rec = work.tile([1, 512], F32, tag="rec", bufs=2,
                                name=f"rec_{h}_{j}")
                nc.vector.reciprocal(rec[:], pav[D:D + 1, :])
                bc = work.tile([64, 512], F32, tag="bc", bufs=3,
                               name=f"bc_{h}_{j}")
                nc.gpsimd.partition_broadcast(bc[:], rec[:])
                nc.vector.tensor_mul(
                    at_s[ho:ho + 64, ht, 512 * j:512 * (j + 1)],
                    pav[0:D, :], bc[:])

            # ---- output projection chunk: y rows [128m,128(m+1)) ----
            def y_chunk(m):
                msl = slice(128 * m, 128 * (m + 1))
                for n in range(2):
                    nsl = slice(512 * n, 512 * (n + 1))
                    py = ps.tile([128, 512], F32, tag="mm",
                                 name=f"py_{m}_{n}")
                    for kk in range(2):
                        nc.tensor.matmul(py[:], (at_s[:, kk, msl]),
                                         (wo_s[:, kk, nsl]),
                                         start=(kk == 0), stop=(kk == 1))
                    ys = work.tile([128, 512], F32, tag="y", bufs=4,
                                   name=f"ys_{m}_{n}")
                    if m >= 12:  # tail rounds: ACT is idle there, DVE is not
                        nc.scalar.copy(ys[:], py[:])
                    else:
                        nc.vector.tensor_copy(ys[:], py[:])
                    nc.sync.dma_start(y[msl, nsl], ys[:])

            # Emission order interleaves phases so ACT (exp) starts as soon as
            # block-0 projections land, and y DMAs spread across all rounds:
            # attention round j needs only qt/kt block 0..j and V chunks
            # i <= 4j+3; y rows 4j..4j+3 need only round j.  Weight DMAs are
            # emitted as late as dataflow allows so x blocks win the queues.
            proj_n(1)
            for k in range(NK):
                nc.sync.dma_start(wv_s[:, k], wvt[128 * k:128 * (k + 1)])
            for m in range(4):
                v_chunk(m)
            attn(0, 0)
            attn(1, 0)
            for m in range(4, 8):
                v_chunk(m)
            xt_dma(2)
            proj_n(2)
            for kk in range(2):
                nc.sync.dma_start(wo_s[:, kk], wot[128 * kk:128 * (kk + 1)])
            attn(2, 0)
            attn(3, 0)
            attn(0, 1)
            attn(1, 1)
            xt_dma(3)
            proj_n(3)
            for m in range(4):
                y_chunk(m)
            attn(2, 1)
            v_chunk(8), v_chunk(9)
            attn(3, 1)
            v_chunk(10), v_chunk(11)
            for m in range(4, 8):
                y_chunk(m)
            attn(0, 2)
            v_chunk(12), v_chunk(13)
            attn(1, 2)
            v_chunk(14), v_chunk(15)
            attn(2, 2)
            attn(3, 2)
            for m in range(8, 12):
                y_chunk(m)
            for h in range(HG):
                attn(h, 3)
            for m in range(12, 16):
                y_chunk(m)
    return nc


_CACHE = {}


def _get_nc():
    if "nc" not in _CACHE:
        nc = bacc.Bacc("TRN2", target_bir_lowering=False, debug=False,
                       enable_asserts=False, num_devices=NCORES)
        build_program(nc)
        nc.compile()
        _CACHE["nc"] = nc
    return _CACHE["nc"]


def make_in_maps(x, wq, wk, wv, wo):
    x = np.asarray(x, dtype=np.float32)
    wq = np.asarray(wq, dtype=np.float32)
    wk = np.asarray(wk, dtype=np.float32)
    wv = np.asarray(wv, dtype=np.float32)
    wo = np.asarray(wo, dtype=np.float32)
    scale = 1.0 / np.sqrt(np.float32(D))
    in_maps = []
    for c in range(NCORES):
        b, g = c // 4, c % 4
        rows = slice(DH * g, DH * (g + 1))
        in_maps.append({
            "xt": np.ascontiguousarray(x[b].T),
            "wqt": np.ascontiguousarray(wq[rows].T * scale),
            "wkt": np.ascontiguousarray(wk[rows].T),
            "wvt": np.ascontiguousarray(wv[rows].T),
            "wot": np.ascontiguousarray(wo[:, rows].T),
        })
    return in_maps


def kernel(x, wq, wk, wv, wo):
    nc = _get_nc()
    in_maps = make_in_maps(x, wq, wk, wv, wo)
    res = bass_utils.run_bass_kernel_spmd(nc, in_maps, core_ids=list(range(NCORES)))
    out = np.empty((B, T, C), dtype=np.float32)
    for b in range(B):
        acc = res.results[4 * b]["y"].astype(np.float32)
        for g in range(1, 4):
            acc = acc + res.results[4 * b + g]["y"]
        out[b] = acc
    return out

